# revision 17
# baseline (speedup 1.0000x reference)
"""Trainium2 Bass kernel for nn_AutoSparseLinear.

Problem: out[b,h,o] = sum_d gathered[b,h,d] * W[h,o,d] + bias[h,o]
  where gathered[b,h,k*64+w] = x[b, mask[h,k], w]
  x: [512,128,64] f32, mask: [256,4] i64, W: [256,64,256] f32, b: [256,64] f32
  out: [512,256,64] f32

Sharding (expert-style per the hint): split H_out 8 ways; each core
computes 32 groups over the full batch B=512.  The host does the
mask-dependent gather in numpy so the device program is identical on
all 8 cores (single SPMD NEFF).

Per-core operands (gathered x split by d-chunk, mixed precision —
chunk 0 in fp8e4m3, chunk 1 in fp16 keeps worst-case rel err ~1.7e-2
< 2e-2 while cutting HBM bytes 25%):
  gx8  [128, 32*512] fp8  — chunk c=0: [p,b] of slot h' = x[b, mask[h, p//64], p%64]
  gx16 [128, 32*512] fp16 — chunk c=1: rows mask[h, 2 + p//64]
  wt   [128, 32*2*64] fp16 — slot(h',c)[p, o] = W[h, o, c*128+p]
  bb   [128, 16] f32 — bias pairs: col j = concat(b[2j], b[2j+1])

Device, per group-pair j (groups 2j, 2j+1 side by side in PE column
tiles): psum[64*hh:64*hh+64, :] = wt(2j+hh,0).T @ gx8(2j+hh)
                                + wt(2j+hh,1).T @ gx16(2j+hh)
(mixed fp8/fp16 operands, fp32 PSUM), then DVE/ACT adds the bias
column and casts to fp16 into an SBUF-resident output chunk; chunks
DMA to DRAM partition-major ([128, 16384]: contiguous per partition).

Schedule notes (from trace analysis):
 - DMA completion semaphores land ~3us after the data (HBM receipt
   round trip), and SDMA engine 15 is a chronic ~1.5x straggler, so
   the tail is where time dies.  Inputs stream on the sync-engine
   HWDGE ring in slice order with compute chasing; the final slice
   goes EARLY on the scalar-engine HWDGE ring so the last couple of
   pairs never waits on the straggler tail.
 - Outputs stage in SBUF and leave as 5 chunk DMAs on the scalar
   ring (final chunk is a single pair to shorten the last drain).
 - Bias-adds alternate DVE / ACT so neither engine gates PSUM reuse.
"""

import numpy as np
import ml_dtypes

import concourse.mybir as mybir
from concourse import bacc
from concourse.tile import TileContext
from concourse.bass_utils import run_bass_kernel_spmd

# Problem shapes (hardcoded per contract)
B = 512
H_IN = 128
W_IN = 64
H_OUT = 256
W_OUT = 64
K = 4
N_CORES = 8
HG = H_OUT // N_CORES  # 32 groups per core
N_PAIRS = HG // 2  # 16
N_SLICES = 8  # gx upload pipelining granularity
GROUPS_PER_SLICE = HG // N_SLICES  # 4
# Pair processing order: pairs 12-15 read the early-uploaded scalar-ring
# slices, so run them BEFORE 10,11 — the critical tail after the last
# sync-ring slice (5) lands is then just pairs 10,11.
PAIR_ORDER = [0, 1, 2, 3, 4, 5, 6, 7, 8, 9, 12, 13, 14, 15, 10, 11]
# output chunking in processing order: (first pair, n pairs); trailing
# chunk small to shorten the final drain
OUT_CHUNKS = [(0, 4), (4, 4), (8, 2), (12, 4), (10, 1), (11, 1)]
# slices uploaded early on the scalar ring (the rest stream on sync)
EARLY_SLICES = (6, 7)
# pairs whose bias-add must stay off the scalar engine (it is busy
# issuing output-chunk DMAs right when the critical tail runs); pair 11
# stays on scalar so the two tail bias-adds run concurrently
VECTOR_BIAS_PAIRS = {10}
# final chunks issued from an otherwise-idle engine each
CHUNK_ISSUE_ENGINE = {4: "scalar", 5: "sync"}

F8 = mybir.dt.float8e4
F16 = mybir.dt.float16
F32 = mybir.dt.float32


def build_nc(loop: int = 1, mode: str = "full", timing: bool = False):
    """Build the (uniform-across-cores) Bass program."""
    nc = bacc.Bacc(None, target_bir_lowering=False)
    gx8_d = nc.dram_tensor("gx8", [128, HG * B], F8, kind="ExternalInput")
    gx16_d = nc.dram_tensor("gx16", [128, HG * B], F16, kind="ExternalInput")
    wt_d = nc.dram_tensor("wt", [128, HG * 2 * W_OUT], F16, kind="ExternalInput")
    bb_d = nc.dram_tensor("bb", [128, N_PAIRS], F32, kind="ExternalInput")
    out_d = nc.dram_tensor("out", [128, N_PAIRS * B], F16, kind="ExternalOutput")

    sl_cols = GROUPS_PER_SLICE * B  # per-slice columns in gx8/gx16 (2048)

    with TileContext(nc) as tc:
        with (
            tc.tile_pool(name="res", bufs=1) as res,
            tc.tile_pool(name="psum", bufs=8, space="PSUM") as psump,
            tc.tile_pool(name="outs", bufs=len(OUT_CHUNKS)) as outp,
        ):

            def uploads():
                # tail slices first, on the scalar HWDGE ring: their packets
                # interleave with the sync ring from t=0 and finish early,
                # so the final pairs never wait on the straggler SDMA tail
                g8s = [None] * N_SLICES
                g16s = [None] * N_SLICES
                for s in EARLY_SLICES:
                    t8 = res.tile([128, sl_cols], F8, tag=f"g8_{s}")
                    nc.scalar.dma_start(
                        out=t8[:], in_=gx8_d[:, s * sl_cols : (s + 1) * sl_cols]
                    )
                    t16 = res.tile([128, sl_cols], F16, tag=f"g16_{s}")
                    nc.scalar.dma_start(
                        out=t16[:], in_=gx16_d[:, s * sl_cols : (s + 1) * sl_cols]
                    )
                    g8s[s] = t8
                    g16s[s] = t16

                wtile = res.tile([128, HG * 2 * W_OUT], F16, tag="wt")
                nc.sync.dma_start(out=wtile[:], in_=wt_d[:, :])
                bt = None
                for s in range(N_SLICES):
                    if s in EARLY_SLICES:
                        continue
                    t8 = res.tile([128, sl_cols], F8, tag=f"g8_{s}")
                    nc.sync.dma_start(
                        out=t8[:], in_=gx8_d[:, s * sl_cols : (s + 1) * sl_cols]
                    )
                    t16 = res.tile([128, sl_cols], F16, tag=f"g16_{s}")
                    nc.sync.dma_start(
                        out=t16[:], in_=gx16_d[:, s * sl_cols : (s + 1) * sl_cols]
                    )
                    g8s[s] = t8
                    g16s[s] = t16
                    if s == 0:
                        # bias is only needed by the first bias-add; keep it
                        # out of the critical wt+slice0 prefix
                        bt = res.tile([128, N_PAIRS], F32, tag="bias")
                        nc.sync.dma_start(out=bt[:], in_=bb_d[:, :])
                return bt, wtile, g8s, g16s

            def compute(bt, wtile, g8s, g16s):
                ob = None
                chunk = {}  # pair j -> (chunk_idx, local_idx, is_last_in_chunk)
                for ci, (p0, np_) in enumerate(OUT_CHUNKS):
                    for jl in range(np_):
                        chunk[p0 + jl] = (ci, jl, jl == np_ - 1)
                for j in PAIR_ORDER:
                    s = (2 * j) // GROUPS_PER_SLICE
                    ci, jl, last_in_chunk = chunk[j]
                    if jl == 0:
                        ob = outp.tile([128, OUT_CHUNKS[ci][1] * B], F16, tag="ob")
                    ps = psump.tile([128, B], F32, tag="ps")
                    for c in range(2):
                        for hh in range(2):  # group 2j+hh -> psum cols 64*hh
                            hloc = (2 * j + hh) - s * GROUPS_PER_SLICE
                            lhsT = wtile[:, ((2 * j + hh) * 2 + c) * W_OUT :][
                                :, :W_OUT
                            ]
                            src = g8s[s] if c == 0 else g16s[s]
                            rhs = src[:, hloc * B : (hloc + 1) * B]
                            nc.tensor.matmul(
                                ps[64 * hh : 64 * hh + 64, :],
                                lhsT,
                                rhs,
                                start=(c == 0),
                                stop=(c == 1),
                            )
                    oslc = ob[:, jl * B : (jl + 1) * B]
                    if j % 2 == 0 or j in VECTOR_BIAS_PAIRS:
                        nc.vector.tensor_scalar_add(oslc, ps[:, :], bt[:, j : j + 1])
                    else:
                        nc.scalar.add(oslc, ps[:, :], bt[:, j : j + 1])
                    if last_in_chunk:
                        c0 = OUT_CHUNKS[ci][0] * B
                        eng = getattr(nc, CHUNK_ISSUE_ENGINE.get(ci, "scalar"))
                        eng.dma_start(
                            out=out_d[:, c0 : c0 + OUT_CHUNKS[ci][1] * B], in_=ob[:]
                        )

            def body(_iv=None):
                args = uploads()
                if mode != "upload":
                    compute(*args)

            if mode == "compute":
                args = uploads()
                if loop > 1:
                    with tc.For_i(0, loop, 1):
                        compute(*args)
                else:
                    compute(*args)
            elif loop > 1:
                with tc.For_i(0, loop, 1):
                    body()
            else:
                body()

    nc.finalize()
    return nc


def shard_inputs(x, mask, W, b):
    """Host-side gather + layout prep. Returns per-core input dicts."""
    x = np.asarray(x, dtype=np.float32)
    mask = np.asarray(mask)
    W = np.asarray(W, dtype=np.float32)
    b = np.asarray(b, dtype=np.float32)

    xT = np.ascontiguousarray(x.transpose(1, 2, 0))  # [i, w, b]
    in_maps = []
    for q in range(N_CORES):
        h0 = q * HG
        mq = mask[h0 : h0 + HG]  # [HG, 4]
        g = xT[mq]  # [HG, 4, 64, B]
        g = g.reshape(HG, 2, 128, B)  # [h', c, p, b]
        gx8 = np.ascontiguousarray(g[:, 0].transpose(1, 0, 2).reshape(128, HG * B))
        gx8 = gx8.astype(ml_dtypes.float8_e4m3fn).view(np.uint8)
        gx16 = np.ascontiguousarray(
            g[:, 1].transpose(1, 0, 2).reshape(128, HG * B)
        ).astype(np.float16)

        Wq = W[h0 : h0 + HG]  # [HG, 64, 256]
        wt = (
            Wq.transpose(0, 2, 1)  # [HG, d, o]
            .reshape(HG, 2, 128, W_OUT)
            .transpose(2, 0, 1, 3)  # [128, HG, 2, o]
            .reshape(128, HG * 2 * W_OUT)
        )
        wt = np.ascontiguousarray(wt).astype(np.float16)

        bb = np.empty((128, N_PAIRS), np.float32)
        for j in range(N_PAIRS):
            bb[:64, j] = b[h0 + 2 * j]
            bb[64:, j] = b[h0 + 2 * j + 1]

        in_maps.append({"gx8": gx8, "gx16": gx16, "wt": wt, "bb": bb})
    return in_maps


def assemble_output(results):
    """results: per-core dicts with 'out' [128, N_PAIRS*B] f16 where
    out[hh*64+o, j*B+b] = out_full[b, h0+2j+hh, o]."""
    out = np.empty((B, H_OUT, W_OUT), np.float32)
    for q, r in enumerate(results):
        a = np.asarray(r["out"], dtype=np.float32).reshape(2, W_OUT, N_PAIRS, B)
        # a[hh, o, j, b] -> [b, j, hh, o]
        out[:, q * HG : (q + 1) * HG, :] = a.transpose(3, 2, 0, 1).reshape(
            B, HG, W_OUT
        )
    return out


_NC_CACHE = {}


def kernel(x, mask, W, b):
    in_maps = shard_inputs(x, mask, W, b)
    if "nc" not in _NC_CACHE:
        _NC_CACHE["nc"] = build_nc()
    nc = _NC_CACHE["nc"]
    res = run_bass_kernel_spmd(nc, in_maps, core_ids=list(range(N_CORES)))
    return assemble_output(res.results)


# revision 19
# speedup vs baseline: 1.0977x; 1.0977x over previous
"""Trainium2 Bass kernel for nn_AutoSparseLinear.

Problem: out[b,h,o] = sum_d gathered[b,h,d] * W[h,o,d] + bias[h,o]
  where gathered[b,h,k*64+w] = x[b, mask[h,k], w]
  x: [512,128,64] f32, mask: [256,4] i64, W: [256,64,256] f32, b: [256,64] f32
  out: [512,256,64] f32

Sharding (expert-style per the hint): split H_out 8 ways; each core
computes 32 groups over the full batch B=512.  The host does the
mask-dependent gather in numpy so the device program is identical on
all 8 cores (single SPMD NEFF).

Per-core operands (gathered x split by d-chunk, mixed precision —
chunk 0 in fp8e4m3, chunk 1 in fp16 keeps worst-case rel err ~1.7e-2
< 2e-2 while cutting HBM bytes 25%):
  gx8  [128, 32*512] fp8  — chunk c=0: [p,b] of slot h' = x[b, mask[h, p//64], p%64]
  gx16 [128, 32*512] fp16 — chunk c=1: rows mask[h, 2 + p//64]
  wt   [128, 32*2*64] fp16 — slot(h',c)[p, o] = W[h, o, c*128+p]
  bb   [128, 16] f32 — bias pairs: col j = concat(b[2j], b[2j+1])

Device, per group-pair j (groups 2j, 2j+1 side by side in PE column
tiles): psum[64*hh:64*hh+64, :] = wt(2j+hh,0).T @ gx8(2j+hh)
                                + wt(2j+hh,1).T @ gx16(2j+hh)
(mixed fp8/fp16 operands, fp32 PSUM), then DVE/ACT adds the bias
column and casts to fp16 into an SBUF-resident output chunk; chunks
DMA to DRAM partition-major ([128, 16384]: contiguous per partition).

Schedule notes (from trace analysis):
 - DMA completion semaphores land ~3us after the data (HBM receipt
   round trip), and SDMA engine 15 is a chronic ~1.5x straggler, so
   the tail is where time dies.  Inputs stream on the sync-engine
   HWDGE ring in slice order with compute chasing; the final slice
   goes EARLY on the scalar-engine HWDGE ring so the last couple of
   pairs never waits on the straggler tail.
 - Outputs stage in SBUF and leave as 5 chunk DMAs on the scalar
   ring (final chunk is a single pair to shorten the last drain).
 - Bias-adds alternate DVE / ACT so neither engine gates PSUM reuse.
"""

import numpy as np
import ml_dtypes

import concourse.mybir as mybir
from concourse import bacc
from concourse.tile import TileContext
from concourse.bass_utils import run_bass_kernel_spmd

# Problem shapes (hardcoded per contract)
B = 512
H_IN = 128
W_IN = 64
H_OUT = 256
W_OUT = 64
K = 4
N_CORES = 8
HG = H_OUT // N_CORES  # 32 groups per core
N_PAIRS = HG // 2  # 16
N_SLICES = 8  # gx upload pipelining granularity
GROUPS_PER_SLICE = HG // N_SLICES  # 4
# Pair processing order: pairs 10-15 read the early-uploaded scalar-ring
# slices, so run them BEFORE 8,9 — the critical tail after the last
# sync-ring slice (4) lands is then just pairs 8,9.
PAIR_ORDER = [0, 1, 2, 3, 4, 5, 6, 7, 10, 11, 12, 13, 14, 15, 8, 9]
# output chunking in processing order: (first pair, n pairs); trailing
# chunk small to shorten the final drain
OUT_CHUNKS = [(0, 4), (4, 4), (10, 2), (12, 4), (8, 2)]
# slices uploaded early on the scalar ring (the rest stream on sync)
EARLY_SLICES = (5, 6, 7)
# pairs whose bias-add must stay off the scalar engine (it is busy
# issuing output-chunk DMAs right when the critical tail runs)
VECTOR_BIAS_PAIRS = {8, 9}
# per-chunk override of the engine that issues the output DMA
CHUNK_ISSUE_ENGINE = {}

F8 = mybir.dt.float8e4
F16 = mybir.dt.float16
F32 = mybir.dt.float32


def build_nc(loop: int = 1, mode: str = "full", timing: bool = False):
    """Build the (uniform-across-cores) Bass program."""
    nc = bacc.Bacc(None, target_bir_lowering=False)
    gx8_d = nc.dram_tensor("gx8", [128, HG * B], F8, kind="ExternalInput")
    gx16_d = nc.dram_tensor("gx16", [128, HG * B], F16, kind="ExternalInput")
    wt_d = nc.dram_tensor("wt", [128, HG * 2 * W_OUT], F16, kind="ExternalInput")
    bb_d = nc.dram_tensor("bb", [128, N_PAIRS], F32, kind="ExternalInput")
    out_d = nc.dram_tensor("out", [128, N_PAIRS * B], F16, kind="ExternalOutput")

    sl_cols = GROUPS_PER_SLICE * B  # per-slice columns in gx8/gx16 (2048)

    with TileContext(nc) as tc:
        with (
            tc.tile_pool(name="res", bufs=1) as res,
            tc.tile_pool(name="psum", bufs=8, space="PSUM") as psump,
            tc.tile_pool(name="outs", bufs=len(OUT_CHUNKS)) as outp,
        ):

            def uploads():
                # tail slices first, on the scalar HWDGE ring: their packets
                # interleave with the sync ring from t=0 and finish early,
                # so the final pairs never wait on the straggler SDMA tail
                g8s = [None] * N_SLICES
                g16s = [None] * N_SLICES
                for s in EARLY_SLICES:
                    t8 = res.tile([128, sl_cols], F8, tag=f"g8_{s}")
                    nc.scalar.dma_start(
                        out=t8[:], in_=gx8_d[:, s * sl_cols : (s + 1) * sl_cols]
                    )
                    t16 = res.tile([128, sl_cols], F16, tag=f"g16_{s}")
                    nc.scalar.dma_start(
                        out=t16[:], in_=gx16_d[:, s * sl_cols : (s + 1) * sl_cols]
                    )
                    g8s[s] = t8
                    g16s[s] = t16

                wtile = res.tile([128, HG * 2 * W_OUT], F16, tag="wt")
                nc.sync.dma_start(out=wtile[:], in_=wt_d[:, :])
                bt = None
                for s in range(N_SLICES):
                    if s in EARLY_SLICES:
                        continue
                    t8 = res.tile([128, sl_cols], F8, tag=f"g8_{s}")
                    nc.sync.dma_start(
                        out=t8[:], in_=gx8_d[:, s * sl_cols : (s + 1) * sl_cols]
                    )
                    t16 = res.tile([128, sl_cols], F16, tag=f"g16_{s}")
                    nc.sync.dma_start(
                        out=t16[:], in_=gx16_d[:, s * sl_cols : (s + 1) * sl_cols]
                    )
                    g8s[s] = t8
                    g16s[s] = t16
                    if s == 0:
                        # bias is only needed by the first bias-add; keep it
                        # out of the critical wt+slice0 prefix
                        bt = res.tile([128, N_PAIRS], F32, tag="bias")
                        nc.sync.dma_start(out=bt[:], in_=bb_d[:, :])
                return bt, wtile, g8s, g16s

            def compute(bt, wtile, g8s, g16s):
                ob = None
                chunk = {}  # pair j -> (chunk_idx, local_idx, is_last_in_chunk)
                for ci, (p0, np_) in enumerate(OUT_CHUNKS):
                    for jl in range(np_):
                        chunk[p0 + jl] = (ci, jl, jl == np_ - 1)
                for j in PAIR_ORDER:
                    s = (2 * j) // GROUPS_PER_SLICE
                    ci, jl, last_in_chunk = chunk[j]
                    if jl == 0:
                        ob = outp.tile([128, OUT_CHUNKS[ci][1] * B], F16, tag="ob")
                    ps = psump.tile([128, B], F32, tag="ps")
                    for c in range(2):
                        for hh in range(2):  # group 2j+hh -> psum cols 64*hh
                            hloc = (2 * j + hh) - s * GROUPS_PER_SLICE
                            lhsT = wtile[:, ((2 * j + hh) * 2 + c) * W_OUT :][
                                :, :W_OUT
                            ]
                            src = g8s[s] if c == 0 else g16s[s]
                            rhs = src[:, hloc * B : (hloc + 1) * B]
                            nc.tensor.matmul(
                                ps[64 * hh : 64 * hh + 64, :],
                                lhsT,
                                rhs,
                                start=(c == 0),
                                stop=(c == 1),
                            )
                    oslc = ob[:, jl * B : (jl + 1) * B]
                    if j % 2 == 0 or j in VECTOR_BIAS_PAIRS:
                        nc.vector.tensor_scalar_add(oslc, ps[:, :], bt[:, j : j + 1])
                    else:
                        nc.scalar.add(oslc, ps[:, :], bt[:, j : j + 1])
                    if last_in_chunk:
                        c0 = OUT_CHUNKS[ci][0] * B
                        eng = getattr(nc, CHUNK_ISSUE_ENGINE.get(ci, "scalar"))
                        eng.dma_start(
                            out=out_d[:, c0 : c0 + OUT_CHUNKS[ci][1] * B], in_=ob[:]
                        )

            def body(_iv=None):
                args = uploads()
                if mode != "upload":
                    compute(*args)

            if mode == "compute":
                args = uploads()
                if loop > 1:
                    with tc.For_i(0, loop, 1):
                        compute(*args)
                else:
                    compute(*args)
            elif loop > 1:
                with tc.For_i(0, loop, 1):
                    body()
            else:
                body()

    nc.finalize()
    return nc


def shard_inputs(x, mask, W, b):
    """Host-side gather + layout prep. Returns per-core input dicts."""
    x = np.asarray(x, dtype=np.float32)
    mask = np.asarray(mask)
    W = np.asarray(W, dtype=np.float32)
    b = np.asarray(b, dtype=np.float32)

    xT = np.ascontiguousarray(x.transpose(1, 2, 0))  # [i, w, b]
    in_maps = []
    for q in range(N_CORES):
        h0 = q * HG
        mq = mask[h0 : h0 + HG]  # [HG, 4]
        g = xT[mq]  # [HG, 4, 64, B]
        g = g.reshape(HG, 2, 128, B)  # [h', c, p, b]
        gx8 = np.ascontiguousarray(g[:, 0].transpose(1, 0, 2).reshape(128, HG * B))
        gx8 = gx8.astype(ml_dtypes.float8_e4m3fn).view(np.uint8)
        gx16 = np.ascontiguousarray(
            g[:, 1].transpose(1, 0, 2).reshape(128, HG * B)
        ).astype(np.float16)

        Wq = W[h0 : h0 + HG]  # [HG, 64, 256]
        wt = (
            Wq.transpose(0, 2, 1)  # [HG, d, o]
            .reshape(HG, 2, 128, W_OUT)
            .transpose(2, 0, 1, 3)  # [128, HG, 2, o]
            .reshape(128, HG * 2 * W_OUT)
        )
        wt = np.ascontiguousarray(wt).astype(np.float16)

        bb = np.empty((128, N_PAIRS), np.float32)
        for j in range(N_PAIRS):
            bb[:64, j] = b[h0 + 2 * j]
            bb[64:, j] = b[h0 + 2 * j + 1]

        in_maps.append({"gx8": gx8, "gx16": gx16, "wt": wt, "bb": bb})
    return in_maps


def assemble_output(results):
    """results: per-core dicts with 'out' [128, N_PAIRS*B] f16 where
    out[hh*64+o, j*B+b] = out_full[b, h0+2j+hh, o]."""
    out = np.empty((B, H_OUT, W_OUT), np.float32)
    for q, r in enumerate(results):
        a = np.asarray(r["out"], dtype=np.float32).reshape(2, W_OUT, N_PAIRS, B)
        # a[hh, o, j, b] -> [b, j, hh, o]
        out[:, q * HG : (q + 1) * HG, :] = a.transpose(3, 2, 0, 1).reshape(
            B, HG, W_OUT
        )
    return out


_NC_CACHE = {}


def kernel(x, mask, W, b):
    in_maps = shard_inputs(x, mask, W, b)
    if "nc" not in _NC_CACHE:
        _NC_CACHE["nc"] = build_nc()
    nc = _NC_CACHE["nc"]
    res = run_bass_kernel_spmd(nc, in_maps, core_ids=list(range(N_CORES)))
    return assemble_output(res.results)
